# revision 52
# baseline (speedup 1.0000x reference)
"""Trainium2 Bass kernel for nn_MultiHeadAttention (channel-attention transformer block).

Math (per batch b, with X* = reshape(*, [C, P]), P = 4096, C = 128, D = 512):
  Q = Xq @ (Wq/temp)^T, K = Xk @ Wk^T, V = Xv @ Wv^T            [C, D]
  per head h (8 heads, ld=64): A_h = softmax(Q_h K_h^T); O_h = A_h V_h
  O = silu(O); O = (O - mean)/(unbiased_std + eps)   (LN affine folded into fc)
  out_pre = (v + Wfc@ln_beta) + O @ (Wfc*ln_gamma)^T
  out = BatchNorm2d(out_pre)   (batch stats over (b,h,w), biased var)

Sharding: data-parallel over batch, 2 batches per core on 8 cores; BatchNorm
statistics combined with a tiny AllReduce ([128,2] per core).  Two warm-up
AllReduces (kernel start + mid) absorb launch skew + keep ncfw hot so the
real AllReduce at the end runs near its latency floor.

v3 notes:
 - batch-pipelined: batch0's projections complete first (weights + b0
   activations DMA'd first); b0's attention runs while b1's projections
   stream; b1's attention interleaves with b0's fc phase.
 - single ACT table set (natural_log_exp): silu computed as x*(1/(1+e^-x))
   with DVE ops; LN/BN rstd via exp(-0.5*ln(var)); Square/Copy/Identity are
   table fillers.  Avoids ~2.7us ACT_TABLE_LOAD thrash per switch.
 - residual added into the fc PSUM via an identity-matmul accumulation step.
 - BN per-channel sums from host-precomputed residual sums + xhatT@colsum(wfc)
   matmuls; only sum-of-squares needs Scalar passes.
"""

import os

import numpy as np

import concourse.mybir as mybir
import concourse.tile as tile
from concourse import bacc
from concourse.bass_utils import run_bass_kernel_spmd
from concourse.masks import make_identity

# ---- problem constants (hardcoded per contract) ----
B, C, HH, WW = 16, 128, 64, 64
P = HH * WW           # 4096
NH, LD = 8, 64
D = NH * LD           # 512
N_CORES = 8
BPC = B // N_CORES    # 2 batches per core
NPC = P // 512        # 8 quad-chunks over contraction / output tiles
LN_EPS = 1e-6
BN_EPS = 1e-5
F32 = mybir.dt.float32
BF16 = mybir.dt.bfloat16
I32 = mybir.dt.int32
FP8 = mybir.dt.float8e4 if os.environ.get('BASS_FP8','1')=='1' else mybir.dt.bfloat16
QSCALE = 4096.0   # host scale on (w_qs/temp): brings +-1/4096 into fp8 range
KVSCALE = 64.0    # host scale on w_ks (K path; V stays bf16 unscaled)

_BUILD_CACHE: dict = {}
LAST_RESULTS = None  # BassKernelResults of the most recent run (for profiling)

SKIP_COLL = os.environ.get("BASS_SKIP_COLL", "0") == "1"
N_WARM_AR = int(os.environ.get("BASS_WARM_AR", "2"))
USE_LNEXP = os.environ.get("BASS_LNEXP", "0") == "1"
MODE = "bf16"  # kept for test harness printing


def _emit(ctx, nc, tc, io):
    AF = mybir.ActivationFunctionType
    ALU = mybir.AluOpType
    AX = mybir.AxisListType

    consts = ctx.enter_context(tc.tile_pool(name="consts", bufs=1))
    wpool = ctx.enter_context(tc.tile_pool(name="wpool", bufs=1))
    fcpool = ctx.enter_context(tc.tile_pool(name="fcpool", bufs=1))
    apool = ctx.enter_context(tc.tile_pool(name="apool", bufs=1))
    a1pool = ctx.enter_context(tc.tile_pool(name="a1pool", bufs=1))
    sb = ctx.enter_context(tc.tile_pool(name="sb", bufs=2))
    big = ctx.enter_context(tc.tile_pool(name="big", bufs=1))
    small = ctx.enter_context(tc.tile_pool(name="small", bufs=4))
    stat = ctx.enter_context(tc.tile_pool(name="stat", bufs=1))
    dram = ctx.enter_context(tc.tile_pool(name="dram", bufs=1, space="DRAM"))

    # ---- warm-up AllReduce #1 at the very top: its trigger only waits for
    # the tiny cwin transfer (~2us head-of-line on gpsimd), and the CC stream
    # gets the barrier+ncfw wakeup out of the way during phase A ----
    if not SKIP_COLL and N_WARM_AR > 0:
        wsb = consts.tile([128, 2], F32, tag="wsb", name="wsb")
        nc.vector.memset(wsb, 0.0)
        cwin = dram.tile([128, 2], F32, tag="cwin", name="cwin")
        cwout = dram.tile([128, 2], F32, tag="cwout", name="cwout")
        nc.gpsimd.dma_start(out=cwin[:, :], in_=wsb)
        nc.gpsimd.collective_compute(
            "AllReduce",
            ALU.add,
            replica_groups=[list(range(N_CORES))],
            ins=[cwin.opt()],
            outs=[cwout.opt()],
        )

    # identity for PE transposes / residual accumulation
    ident_f = consts.tile([128, 128], F32, tag="identf", name="identf")
    make_identity(nc, ident_f)
    ident = consts.tile([128, 128], BF16, tag="ident", name="ident")
    nc.vector.tensor_copy(out=ident, in_=ident_f)

    # ---- phase A0 DMAs: 2 groups x 16 contraction slices; 4-8KB/partition
    # transfers keep the DMA packets large ----
    NG, JG = 4, 8
    wts = []   # held weight group tiles, reused for batch1
    a0 = []    # batch0 activation group tiles
    for g in range(NG):
        wq_c = wpool.tile([128, JG, D], FP8, tag=f"wq{g}", name=f"wq{g}")
        wk_c = wpool.tile([128, JG, D], FP8, tag=f"wk{g}", name=f"wk{g}")
        wv_c = wpool.tile([128, JG, D], BF16, tag=f"wv{g}", name=f"wv{g}")
        nc.sync.dma_start(out=wq_c, in_=io["wq"][g])
        nc.scalar.dma_start(out=wk_c, in_=io["wk"][g])
        nc.gpsimd.dma_start(out=wv_c, in_=io["wv"][g])
        wts.append((wq_c, wk_c, wv_c))
        qc = apool.tile([128, JG, 128], FP8, tag=f"qc0_{g}", name="qc0")
        kc = apool.tile([128, JG, 128], FP8, tag=f"kc0_{g}", name="kc0")
        vc = apool.tile([128, JG, 128], BF16, tag=f"vc0_{g}", name="vc0")
        nc.sync.dma_start(out=qc, in_=io["qT"][0, g])
        nc.scalar.dma_start(out=kc, in_=io["kT"][0, g])
        nc.gpsimd.dma_start(out=vc, in_=io["vT"][0, g])
        a0.append((qc, kc, vc))

    # ================= phase A0: batch0 projections =================
    ps_proj0_ctx = tc.tile_pool(name="ps_proj0", bufs=1, space="PSUM")
    ps_proj0 = ps_proj0_ctx.__enter__()
    warm = ps_proj0.tile([128, 128], BF16, tag="warm", name="warm")
    nc.tensor.transpose(warm[:, :], ident[:, :], ident[:, :])
    Qp0 = ps_proj0.tile([128, D], F32, tag="Qp0", name="Qp0")
    Kp0 = ps_proj0.tile([128, D], F32, tag="Kp0", name="Kp0")
    Vp0 = ps_proj0.tile([128, D], F32, tag="Vp0", name="Vp0")

    for g in range(NG):
        qc, kc, vc = a0[g]
        wq_c, wk_c, wv_c = wts[g]
        for j in range(JG):
            st = g == 0 and j == 0
            sp = g == NG - 1 and j == JG - 1
            nc.tensor.matmul(Qp0[:, :], qc[:, j, :], wq_c[:, j, :], start=st, stop=sp)
            nc.tensor.matmul(Kp0[:, :], kc[:, j, :], wk_c[:, j, :], start=st, stop=sp)
            nc.tensor.matmul(Vp0[:, :], vc[:, j, :], wv_c[:, j, :], start=st, stop=sp)

    # batch1 activation streams: all on the sync queue, after the prefetches.
    # The a1pool rotation gives the later descgens WAR waits; sync's only
    # later work is the output stores, so the blocking is harmless there.
    a1 = []
    for g in range(NG):
        qc = a1pool.tile([128, JG, 128], FP8, tag=f"qc1_{g}", name="qc1")
        kc = a1pool.tile([128, JG, 128], FP8, tag=f"kc1_{g}", name="kc1")
        vc = a1pool.tile([128, JG, 128], BF16, tag=f"vc1_{g}", name="vc1")
        nc.sync.dma_start(out=qc, in_=io["qT"][1, g])
        nc.sync.dma_start(out=kc, in_=io["kT"][1, g])
        nc.sync.dma_start(out=vc, in_=io["vT"][1, g])
        a1.append((qc, kc, vc))

    # fc weights / residual / stats constants stream right after phase A0
    # (scalar/gpsimd queues; none of these DMAs carries a WAR wait, so the
    # issuing engines never block on them)
    wfcts = []
    for pt in range(NPC):
        wfct = fcpool.tile([128, 4, 512], BF16, tag=f"wfct{pt}", name=f"wfct{pt}")
        (nc.sync if pt % 2 == 0 else nc.gpsimd).dma_start(out=wfct, in_=io["wfc"][pt])
        wfcts.append(wfct)
    resid = []
    for b in range(BPC):
        t = fcpool.tile([128, NPC, 512], BF16, tag=f"resid{b}", name=f"resid{b}")
        (nc.sync if b == 0 else nc.gpsimd).dma_start(out=t, in_=io["resid"][b])
        resid.append(t)
    colsum = consts.tile([128, 4, NPC], BF16, tag="colsum", name="colsum")
    nc.gpsimd.dma_start(out=colsum, in_=io["colsum"][:, :, :])
    vsums = consts.tile([128, 2 * NPC], F32, tag="vsums", name="vsums")
    nc.gpsimd.dma_start(out=vsums, in_=io["vsums"][:, :])
    bng = consts.tile([128, 1], F32, tag="bng", name="bng")
    bnb = consts.tile([128, 1], F32, tag="bnb", name="bnb")
    nc.gpsimd.dma_start(out=bng, in_=io["bng"][:, :])
    nc.gpsimd.dma_start(out=bnb, in_=io["bnb"][:, :])

    # ---- evacuate batch0 Q/K/V, free proj0 banks ----
    Q_sb0 = sb.tile([128, D], BF16, tag="Q_sb", name="Q_sb0")
    K_sb0 = sb.tile([128, D], BF16, tag="K_sb", name="K_sb0")
    V_sb0 = sb.tile([128, D], BF16, tag="V_sb", name="V_sb0")
    nc.vector.tensor_copy(out=Q_sb0, in_=Qp0[:, :])
    nc.scalar.copy(out=K_sb0, in_=Kp0[:, :])
    nc.vector.tensor_copy(out=V_sb0, in_=Vp0[:, :])
    ps_proj0_ctx.__exit__(None, None, None)

    # attention-era PSUM (sm x3 + O0/O1) + batch1 projection banks (x3) = 8
    ps_s = ctx.enter_context(tc.tile_pool(name="ps_s", bufs=3, space="PSUM"))
    ps_o = ctx.enter_context(tc.tile_pool(name="ps_o", bufs=1, space="PSUM"))
    ps_proj1_ctx = tc.tile_pool(name="ps_proj1", bufs=1, space="PSUM")
    ps_proj1 = ps_proj1_ctx.__enter__()
    Qp1 = ps_proj1.tile([128, D], F32, tag="Qp1", name="Qp1")
    Kp1 = ps_proj1.tile([128, D], F32, tag="Kp1", name="Kp1")
    Vp1 = ps_proj1.tile([128, D], F32, tag="Vp1", name="Vp1")

    Opsums = [ps_o.tile([128, D], F32, tag=f"O{b}", name=f"O{b}") for b in range(BPC)]
    Oscs = [big.tile([128, D], F32, tag=f"Osc{b}", name=f"Osc{b}") for b in range(BPC)]

    def qk_transposes(Q_sb, K_sb, QT_sb, KT_sb):
        # copies all on Vector: Scalar's queue carries Exp/silu chains whose
        # latency must not gate the next batch's S-matmuls
        for src, dst in ((Q_sb, QT_sb), (K_sb, KT_sb)):
            for dc in range(4):
                tp = ps_s.tile([128, 128], BF16, tag="sm", name="stp")
                nc.tensor.transpose(tp[:, :], src[:, dc * 128:(dc + 1) * 128], ident[:, :])
                nc.vector.tensor_copy(out=dst[:, dc * 128:(dc + 1) * 128], in_=tp[:, :])

    def start_head(b, h, QT_sb, KT_sb):
        """Issue S-matmul + Exp for head h (finished one iteration later, so
        the Exp latency hides under the next head's S + filler PE work)."""
        po = (h % 2) * 64
        fo = (h // 2) * 128
        S = ps_s.tile([128, 128], F32, tag="sm", name="S")
        nc.tensor.matmul(S[:, :], QT_sb[po:po + 64, fo:fo + 128],
                         KT_sb[po:po + 64, fo:fo + 128], start=True, stop=True)
        e_f = sb.tile([128, 128], BF16, tag="e_f", name="e_f")
        lsum = small.tile([128, 1], F32, tag="lsum", name="lsum")
        nc.scalar.activation(out=e_f, in_=S[:, :], func=AF.Exp,
                             scale=1.0 / (QSCALE * KVSCALE), accum_out=lsum)
        rs = small.tile([128, 1], F32, tag="rs", name="rs")
        nc.vector.reciprocal(rs, lsum)
        return (h, e_f, rs)

    def finish_head(b, st, V_sb):
        h, e_f, rs = st
        tpa = ps_s.tile([128, 128], BF16, tag="sm", name="stp")
        nc.tensor.transpose(tpa[:, :], e_f[:, :], ident[:, :])
        aT = sb.tile([128, 128], BF16, tag="aT", name="aT")
        nc.vector.tensor_copy(out=aT, in_=tpa[:, :])
        nc.tensor.matmul(Opsums[b][:, h * 64:(h + 1) * 64], aT[:, :],
                         V_sb[:, h * 64:(h + 1) * 64], start=True, stop=True)
        nc.vector.tensor_scalar_mul(out=Oscs[b][:, h * 64:(h + 1) * 64],
                                    in0=Opsums[b][:, h * 64:(h + 1) * 64],
                                    scalar1=rs)

    def head_loop(b, QT_sb, KT_sb, V_sb, filler):
        pending = None
        for h in range(NH):
            st = start_head(b, h, QT_sb, KT_sb)
            if filler is not None:
                filler(h)
            if pending is not None:
                finish_head(b, pending, V_sb)
            pending = st
        finish_head(b, pending, V_sb)

    # ---- batch0 attention, with batch1 projections as PE filler ----
    QT0 = sb.tile([128, D], BF16, tag="QT_sb", name="QT0")
    KT0 = sb.tile([128, D], BF16, tag="KT_sb", name="KT0")
    qk_transposes(Q_sb0, K_sb0, QT0, KT0)

    def proj1_filler(h):
        g = h // 2
        qc, kc, vc = a1[g]
        wq_c, wk_c, wv_c = wts[g]
        for j in range((h % 2) * 4, (h % 2) * 4 + 4):
            st = h == 0 and j == 0
            sp = h == NH - 1 and j == JG - 1
            nc.tensor.matmul(Qp1[:, :], qc[:, j, :], wq_c[:, j, :], start=st, stop=sp)
            nc.tensor.matmul(Kp1[:, :], kc[:, j, :], wk_c[:, j, :], start=st, stop=sp)
            nc.tensor.matmul(Vp1[:, :], vc[:, j, :], wv_c[:, j, :], start=st, stop=sp)

    head_loop(0, QT0, KT0, V_sb0, proj1_filler)

    # ---- evacuate batch1 Q/K/V, free proj1 banks; open fc-era pools ----
    Q_sb1 = sb.tile([128, D], BF16, tag="Q_sb", name="Q_sb1")
    K_sb1 = sb.tile([128, D], BF16, tag="K_sb", name="K_sb1")
    V_sb1 = sb.tile([128, D], BF16, tag="V_sb", name="V_sb1")
    nc.vector.tensor_copy(out=Q_sb1, in_=Qp1[:, :])
    nc.scalar.copy(out=K_sb1, in_=Kp1[:, :])
    nc.vector.tensor_copy(out=V_sb1, in_=Vp1[:, :])
    ps_proj1_ctx.__exit__(None, None, None)
    ps_fc = ctx.enter_context(tc.tile_pool(name="ps_fc", bufs=2, space="PSUM"))
    ps_fs = ctx.enter_context(tc.tile_pool(name="ps_fs", bufs=1, space="PSUM"))
    fcS = ps_fs.tile([128, 2 * NPC], F32, tag="fcS", name="fcS")

    def rsqrt_dve(dst, src, uid, iters=2):
        """1/sqrt(src) on the Vector engine: bit-trick seed + Newton iters.
        No Scalar sqrt -> no ACT table-set switch. rel err ~4e-6 (2 it),
        ~1.8e-3 (1 it)."""
        xi = small.tile([128, 1], I32, tag="nxi", name="nxi")
        nc.vector.tensor_scalar(out=xi, in0=src.bitcast(I32), scalar1=1,
                                scalar2=None, op0=ALU.logical_shift_right)
        yi = small.tile([128, 1], I32, tag="nyi", name="nyi")
        nc.vector.tensor_scalar(out=yi, in0=xi, scalar1=-1, scalar2=0x5F3759DF,
                                op0=ALU.mult, op1=ALU.add)
        y = small.tile([128, 1], F32, tag="ny0", name="ny0")
        nc.vector.tensor_copy(out=y, in_=yi.bitcast(F32))
        for it in range(iters):
            t = small.tile([128, 1], F32, tag=f"nt{it}", name="nt")
            nc.vector.tensor_mul(out=t, in0=y, in1=y)
            nc.vector.tensor_mul(out=t, in0=t, in1=src)
            nc.vector.tensor_scalar(out=t, in0=t, scalar1=-0.5, scalar2=1.5,
                                    op0=ALU.mult, op1=ALU.add)
            yn = dst if it == iters - 1 else small.tile([128, 1], F32, tag="ny1", name="ny1")
            nc.vector.tensor_mul(out=yn, in0=y, in1=t)
            y = yn

    def silu_ln_xhat(b):
        """silu (exp + DVE) -> LayerNorm -> xhat (no PE work)."""
        Osc = Oscs[b]
        # silu(x) = x * 0.5*(1 + tanh(x/2)); Tanh shares the exp ACT table
        # set (no reload) and the elementwise ops run on the idle GpSimd
        e1 = big.tile([128, D], F32, tag="e1", name="e1", bufs=2)
        nc.scalar.activation(out=e1, in_=Osc, func=AF.Tanh, scale=0.5)
        nc.vector.tensor_scalar(out=e1, in0=e1, scalar1=0.5, scalar2=0.5,
                                op0=ALU.mult, op1=ALU.add)
        nc.vector.tensor_mul(out=Osc, in0=Osc, in1=e1)   # Osc <- silu(Osc)
        st6 = small.tile([128, 6], F32, tag="st6", name="st6")
        nc.vector.bn_stats(out=st6, in_=Osc)
        mv = small.tile([128, 2], F32, tag="mv", name="mv")
        nc.vector.bn_aggr(out=mv, in_=st6)
        vu = small.tile([128, 1], F32, tag="vu", name="vu")
        nc.vector.tensor_scalar_mul(out=vu, in0=mv[:, 1:2], scalar1=float(D) / (D - 1))
        rstd = small.tile([128, 1], F32, tag="rstd", name="rstd")
        rsqrt_dve(rstd, vu, b)
        xhat = sb.tile([128, D], BF16, tag="xhat", name="xhat")
        nc.vector.tensor_scalar(out=xhat, in0=Osc, scalar1=mv[:, 0:1], scalar2=rstd,
                                op0=ALU.subtract, op1=ALU.mult)
        return xhat

    def xt_fcs(b, xhat):
        """xhat^T via PE transposes + the per-pt fc colsum matmuls."""
        xT = sb.tile([128, D], BF16, tag="xT", name="xT")
        for dc in range(4):
            tp = ps_s.tile([128, 128], BF16, tag="sm", name="stp")
            nc.tensor.transpose(tp[:, :], xhat[:, dc * 128:(dc + 1) * 128], ident[:, :])
            nc.vector.tensor_copy(out=xT[:, dc * 128:(dc + 1) * 128], in_=tp[:, :])
        for dc in range(4):
            nc.tensor.matmul(fcS[:, b * NPC:(b + 1) * NPC],
                             xT[:, dc * 128:(dc + 1) * 128],
                             colsum[:, dc, :], start=dc == 0, stop=dc == 3)
        return xT

    segs = [big.tile([128, NPC, 512], BF16, tag=f"seg{b}", name=f"seg{b}")
            for b in range(BPC)]
    pcols = stat.tile([128, 16], F32, tag="pcols", name="pcols")

    def fc_pt(b, pt, xT):
        """fc for one 512-pixel tile + residual + sumsq accumulation."""
        O2 = ps_fc.tile([128, 512], F32, tag="O2", name="O2")
        for dc in range(4):
            nc.tensor.matmul(O2[:, :], xT[:, dc * 128:(dc + 1) * 128],
                             wfcts[pt][:, dc, :], start=dc == 0, stop=False)
        nc.tensor.matmul(O2[:, :], ident[:, :], resid[b][:, pt, :],
                         start=False, stop=True)
        seg = segs[b][:, pt, :]
        if pt % 2 == 0:
            nc.vector.tensor_copy(out=seg, in_=O2[:, :])
        else:
            nc.scalar.copy(out=seg, in_=O2[:, :])
        junk = sb.tile([128, 512], BF16, tag="junk", name="junk")
        nc.scalar.activation(out=junk, in_=O2[:, :], func=AF.Square,
                             accum_out=pcols[:, b * NPC + pt:b * NPC + pt + 1])

    # ---- batch1 Q/K transposes first (PE/Vector free), then batch0's
    # silu/LN chain (Scalar/DVE) overlaps batch1's first attention heads;
    # batch0's xT generation and fc tiles fill the PE inside the head loop ----
    QT1 = sb.tile([128, D], BF16, tag="QT_sb", name="QT1")
    KT1 = sb.tile([128, D], BF16, tag="KT_sb", name="KT1")
    qk_transposes(Q_sb1, K_sb1, QT1, KT1)
    xhat0 = silu_ln_xhat(0)
    xT0_box = []

    def fc0_filler(h):
        if h == 0:
            xT0_box.append(xt_fcs(0, xhat0))
        elif h <= 5:
            fc_pt(0, h - 1, xT0_box[0])

    head_loop(1, QT1, KT1, V_sb1, fc0_filler)

    # ---- batch1 silu/LN/xT + fc; batch0's last fc tiles keep the PE fed
    # while batch1's LN chain runs on Scalar/Vector ----
    xhat1 = silu_ln_xhat(1)
    for pt in range(5, NPC):
        fc_pt(0, pt, xT0_box[0])
    xT1 = xt_fcs(1, xhat1)
    cin = dram.tile([128, 2], F32, tag="cin", name="cin")
    cout = dram.tile([128, 2], F32, tag="cout", name="cout")
    stats2 = stat.tile([128, 2], F32, tag="stats2", name="stats2")
    ssum = stat.tile([128, NPC], F32, tag="ssum", name="ssum")
    nc.vector.tensor_add(out=ssum, in0=vsums[:, 0:NPC], in1=vsums[:, NPC:2 * NPC])
    nc.vector.tensor_add(out=ssum, in0=ssum, in1=fcS[:, 0:NPC])
    nc.vector.tensor_add(out=ssum, in0=ssum, in1=fcS[:, NPC:2 * NPC])
    nc.vector.reduce_sum(stats2[:, 0:1], ssum, axis=AX.X)
    nc.gpsimd.dma_start(out=cin[:, 0:1], in_=stats2[:, 0:1])
    for pt in range(NPC):
        fc_pt(1, pt, xT1)

    # second warm AllReduce, emitted HERE so its trigger (which blocks the
    # gpsimd queue until warm #1 completes) sits after every descgen the
    # compute phases need; it still executes mid-kernel, re-syncing cores
    for _ in range(max(0, N_WARM_AR - 1) if not SKIP_COLL else 0):
        nc.gpsimd.collective_compute(
            "AllReduce",
            ALU.add,
            replica_groups=[list(range(N_CORES))],
            ins=[cwin.opt()],
            outs=[cwout.opt()],
        )

    # ---- BN stats AllReduce + normalize + store ----
    # sums side computed + shipped as soon as both fcS halves exist; only the
    # sum-of-squares reduce gates the AllReduce trigger
    nc.vector.reduce_sum(stats2[:, 1:2], pcols[:, 0:16], axis=AX.X)
    nc.gpsimd.dma_start(out=cin[:, 1:2], in_=stats2[:, 1:2])
    if SKIP_COLL:
        nc.gpsimd.dma_start(out=cout[:, :], in_=cin[:, :])
    else:
        nc.gpsimd.collective_compute(
            "AllReduce",
            ALU.add,
            replica_groups=[list(range(N_CORES))],
            ins=[cin.opt()],
            outs=[cout.opt()],
        )
    red = stat.tile([128, 2], F32, tag="red", name="red")
    nc.gpsimd.dma_start(out=red[:, :], in_=cout[:, :])

    inv_n = 1.0 / float(B * P)
    me = stat.tile([128, 2], F32, tag="me", name="me")  # [mean, E[x^2]]
    nc.vector.tensor_scalar_mul(out=me, in0=red[:, :], scalar1=inv_n)
    msq = small.tile([128, 1], F32, tag="msq", name="msq")
    nc.vector.tensor_mul(out=msq, in0=me[:, 0:1], in1=me[:, 0:1])
    var = small.tile([128, 1], F32, tag="var", name="var")
    nc.vector.tensor_sub(out=var, in0=me[:, 1:2], in1=msq)
    nc.vector.tensor_scalar_add(out=var, in0=var, scalar1=BN_EPS)
    invs = small.tile([128, 1], F32, tag="invs", name="invs")
    rsqrt_dve(invs, var, 99, iters=1)
    scl = small.tile([128, 1], F32, tag="scl", name="scl")
    nc.vector.tensor_mul(out=scl, in0=bng, in1=invs)
    tmp = small.tile([128, 1], F32, tag="tmp", name="tmp")
    nc.vector.tensor_mul(out=tmp, in0=me[:, 0:1], in1=scl)
    shf = small.tile([128, 1], F32, tag="shf", name="shf")
    nc.vector.tensor_sub(out=shf, in0=bnb, in1=tmp)

    # normalize all segments (Vector/Scalar split), then one large
    # contiguous store per batch (8KB/partition -> big DMA packets)
    for b in range(BPC):
        for half in range(2):
            for pt in range(half * 4, half * 4 + 4):
                seg = segs[b][:, pt, :]
                if pt % 2 == 0:
                    nc.vector.tensor_scalar(out=seg, in0=seg, scalar1=scl, scalar2=shf,
                                            op0=ALU.mult, op1=ALU.add)
                else:
                    nc.scalar.activation(out=seg, in_=seg, func=AF.Identity,
                                         scale=scl, bias=shf)
            (nc.sync if (b + half) % 2 == 0 else nc.gpsimd).dma_start(
                out=io["out"][b, :, half * 2048:(half + 1) * 2048],
                in_=segs[b][:, half * 4:half * 4 + 4, :])


def _build():
    key = (SKIP_COLL, N_WARM_AR, USE_LNEXP)
    if key in _BUILD_CACHE:
        return _BUILD_CACHE[key]
    nc = bacc.Bacc("TRN2", target_bir_lowering=False, debug=False, num_devices=N_CORES)
    io = {
        "qT": nc.dram_tensor("qT", [BPC, 4, 128, 8, 128], FP8, kind="ExternalInput").ap(),
        "kT": nc.dram_tensor("kT", [BPC, 4, 128, 8, 128], FP8, kind="ExternalInput").ap(),
        "vT": nc.dram_tensor("vT", [BPC, 4, 128, 8, 128], BF16, kind="ExternalInput").ap(),
        "resid": nc.dram_tensor("resid", [BPC, C, NPC, 512], BF16, kind="ExternalInput").ap(),
        "wq": nc.dram_tensor("wq", [4, 128, 8, D], FP8, kind="ExternalInput").ap(),
        "wk": nc.dram_tensor("wk", [4, 128, 8, D], FP8, kind="ExternalInput").ap(),
        "wv": nc.dram_tensor("wv", [4, 128, 8, D], BF16, kind="ExternalInput").ap(),
        "wfc": nc.dram_tensor("wfc", [NPC, 128, 4, 512], BF16, kind="ExternalInput").ap(),
        "colsum": nc.dram_tensor("colsum", [128, 4, NPC], BF16, kind="ExternalInput").ap(),
        "vsums": nc.dram_tensor("vsums", [C, 2 * NPC], F32, kind="ExternalInput").ap(),
        "bng": nc.dram_tensor("bng", [C, 1], F32, kind="ExternalInput").ap(),
        "bnb": nc.dram_tensor("bnb", [C, 1], F32, kind="ExternalInput").ap(),
        "out": nc.dram_tensor("out", [BPC, C, P], BF16, kind="ExternalOutput").ap(),
    }
    from contextlib import ExitStack
    with tile.TileContext(nc) as tc, ExitStack() as ctx:
        _emit(ctx, nc, tc, io)
    nc.compile()
    _BUILD_CACHE[key] = nc
    return nc


def _bf16(x):
    import ml_dtypes
    return np.ascontiguousarray(np.asarray(x, np.float32).astype(ml_dtypes.bfloat16))


def _fp8(x):
    import ml_dtypes
    dt = ml_dtypes.float8_e4m3fn if os.environ.get('BASS_FP8','1')=='1' else ml_dtypes.bfloat16
    return np.ascontiguousarray(np.asarray(x, np.float32).astype(dt))


def _pack_acts(xT):
    # [b, 4096, 128] -> [b, 4, 128, 8, 128]  (group, partition, slice, c)
    b = xT.shape[0]
    return np.ascontiguousarray(
        xT.reshape(b, 4, 8, 128, 128).transpose(0, 1, 3, 2, 4))


def _pack_w(w):
    # [4096, D] -> [4, 128, 8, D]
    return np.ascontiguousarray(w.reshape(4, 8, 128, -1).transpose(0, 2, 1, 3))


def kernel(v, k, q, w_qs, w_ks, w_vs, w_fc, ln_gamma, ln_beta, temperature,
           bn_gamma, bn_beta, **_ignored):
    v = np.asarray(v, np.float32)
    k = np.asarray(k, np.float32)
    q = np.asarray(q, np.float32)
    w_qs = np.asarray(w_qs, np.float32)
    w_ks = np.asarray(w_ks, np.float32)
    w_vs = np.asarray(w_vs, np.float32)
    w_fc = np.asarray(w_fc, np.float32)
    ln_gamma = np.asarray(ln_gamma, np.float32)
    ln_beta = np.asarray(ln_beta, np.float32)
    temp = float(np.asarray(temperature))
    bn_gamma = np.asarray(bn_gamma, np.float32)
    bn_beta = np.asarray(bn_beta, np.float32)

    qf = q.reshape(B, C, P)
    kf = k.reshape(B, C, P)
    vf = v.reshape(B, C, P)
    qT = _fp8(_pack_acts(qf.transpose(0, 2, 1)))
    kT = _fp8(_pack_acts(kf.transpose(0, 2, 1)))
    vT = _bf16(_pack_acts(vf.transpose(0, 2, 1)))
    wq = _fp8(_pack_w((w_qs * (QSCALE / temp)).T))
    wk = _fp8(_pack_w((w_ks * KVSCALE).T))
    wv = _bf16(_pack_w(w_vs.T))
    # wfc packed as [pt, d_low, dc, p_in_pt]: wfcT_eff[dc*128+d_low, pt*512+p]
    wfcT_eff = (w_fc * ln_gamma[None, :]).T  # [D, P]
    wfc = _bf16(wfcT_eff.reshape(4, 128, NPC, 512).transpose(2, 1, 0, 3))
    # colsum[d_low, dc, pt] = sum_{p in pt} wfcT_eff[dc*128+d_low, pt*512+p]
    colsum = _bf16(wfcT_eff.reshape(4, 128, NPC, 512).sum(-1).transpose(1, 0, 2))
    bias_fc = (w_fc @ ln_beta).astype(np.float32)
    veff = vf + bias_fc[None, None, :]                      # [B, C, P] f32
    resid = _bf16(veff.reshape(B, C, NPC, 512))
    # per-(b,pt) channel sums of the (bf16-rounded) residual, f32 accumulated
    vsums_full = np.asarray(resid, np.float32).sum(-1)      # [B, C, NPC]
    bng = np.ascontiguousarray(bn_gamma.reshape(C, 1))
    bnb = np.ascontiguousarray(bn_beta.reshape(C, 1))

    nc = _build()
    in_maps = []
    for i in range(N_CORES):
        bs = slice(BPC * i, BPC * (i + 1))
        vsums = np.ascontiguousarray(
            vsums_full[bs].transpose(1, 0, 2).reshape(C, 2 * NPC))
        in_maps.append({
            "qT": qT[bs], "kT": kT[bs], "vT": vT[bs], "resid": resid[bs],
            "wq": wq, "wk": wk, "wv": wv, "wfc": wfc,
            "colsum": colsum, "vsums": vsums,
            "bng": bng, "bnb": bnb,
        })
    res = run_bass_kernel_spmd(nc, in_maps, core_ids=list(range(N_CORES)))
    global LAST_RESULTS
    LAST_RESULTS = res
    out = np.concatenate([np.asarray(res.results[i]["out"], np.float32)
                          for i in range(N_CORES)], axis=0)   # [B, C, P]
    return out.reshape(B, C, HH, WW)


# revision 56
# speedup vs baseline: 1.0408x; 1.0408x over previous
"""Trainium2 Bass kernel for nn_MultiHeadAttention (channel-attention transformer block).

Math (per batch b, with X* = reshape(*, [C, P]), P = 4096, C = 128, D = 512):
  Q = Xq @ (Wq/temp)^T, K = Xk @ Wk^T, V = Xv @ Wv^T            [C, D]
  per head h (8 heads, ld=64): A_h = softmax(Q_h K_h^T); O_h = A_h V_h
  O = silu(O); O = (O - mean)/(unbiased_std + eps)   (LN affine folded into fc)
  out_pre = (v + Wfc@ln_beta) + O @ (Wfc*ln_gamma)^T
  out = BatchNorm2d(out_pre)   (batch stats over (b,h,w), biased var)

Sharding: data-parallel over batch, 2 batches per core on 8 cores; BatchNorm
statistics combined with a tiny AllReduce ([128,2] per core).  Two warm-up
AllReduces (kernel start + mid) absorb launch skew + keep ncfw hot so the
real AllReduce at the end runs near its latency floor.

v3 notes:
 - batch-pipelined: batch0's projections complete first (weights + b0
   activations DMA'd first); b0's attention runs while b1's projections
   stream; b1's attention interleaves with b0's fc phase.
 - single ACT table set (natural_log_exp): silu computed as x*(1/(1+e^-x))
   with DVE ops; LN/BN rstd via exp(-0.5*ln(var)); Square/Copy/Identity are
   table fillers.  Avoids ~2.7us ACT_TABLE_LOAD thrash per switch.
 - residual added into the fc PSUM via an identity-matmul accumulation step.
 - BN per-channel sums from host-precomputed residual sums + xhatT@colsum(wfc)
   matmuls; only sum-of-squares needs Scalar passes.
"""

import os

import numpy as np

import concourse.mybir as mybir
import concourse.tile as tile
from concourse import bacc
from concourse.bass_utils import run_bass_kernel_spmd
from concourse.masks import make_identity

# ---- problem constants (hardcoded per contract) ----
B, C, HH, WW = 16, 128, 64, 64
P = HH * WW           # 4096
NH, LD = 8, 64
D = NH * LD           # 512
N_CORES = 8
BPC = B // N_CORES    # 2 batches per core
NPC = P // 512        # 8 quad-chunks over contraction / output tiles
LN_EPS = 1e-6
BN_EPS = 1e-5
F32 = mybir.dt.float32
BF16 = mybir.dt.bfloat16
I32 = mybir.dt.int32
FP8 = mybir.dt.float8e4 if os.environ.get('BASS_FP8','1')=='1' else mybir.dt.bfloat16
QSCALE = 4096.0   # host scale on (w_qs/temp): brings +-1/4096 into fp8 range
KVSCALE = 64.0    # host scale on w_ks (K path; V stays bf16 unscaled)

_BUILD_CACHE: dict = {}
LAST_RESULTS = None  # BassKernelResults of the most recent run (for profiling)

SKIP_COLL = os.environ.get("BASS_SKIP_COLL", "0") == "1"
N_WARM_AR = int(os.environ.get("BASS_WARM_AR", "2"))
USE_LNEXP = os.environ.get("BASS_LNEXP", "0") == "1"
MODE = "bf16"  # kept for test harness printing


def _emit(ctx, nc, tc, io):
    AF = mybir.ActivationFunctionType
    ALU = mybir.AluOpType
    AX = mybir.AxisListType

    consts = ctx.enter_context(tc.tile_pool(name="consts", bufs=1))
    wpool = ctx.enter_context(tc.tile_pool(name="wpool", bufs=1))
    fcpool = ctx.enter_context(tc.tile_pool(name="fcpool", bufs=1))
    apool = ctx.enter_context(tc.tile_pool(name="apool", bufs=1))
    a1pool = ctx.enter_context(tc.tile_pool(name="a1pool", bufs=1))
    sb = ctx.enter_context(tc.tile_pool(name="sb", bufs=2))
    big = ctx.enter_context(tc.tile_pool(name="big", bufs=1))
    small = ctx.enter_context(tc.tile_pool(name="small", bufs=4))
    stat = ctx.enter_context(tc.tile_pool(name="stat", bufs=1))
    dram = ctx.enter_context(tc.tile_pool(name="dram", bufs=1, space="DRAM"))

    # ---- warm-up AllReduce #1 at the very top: its trigger only waits for
    # the tiny cwin transfer (~2us head-of-line on gpsimd), and the CC stream
    # gets the barrier+ncfw wakeup out of the way during phase A ----
    if not SKIP_COLL and N_WARM_AR > 0:
        wsb = consts.tile([128, 2], F32, tag="wsb", name="wsb")
        nc.vector.memset(wsb, 0.0)
        cwin = dram.tile([128, 2], F32, tag="cwin", name="cwin")
        cwout = dram.tile([128, 2], F32, tag="cwout", name="cwout")
        nc.gpsimd.dma_start(out=cwin[:, :], in_=wsb)
        nc.gpsimd.collective_compute(
            "AllReduce",
            ALU.add,
            replica_groups=[list(range(N_CORES))],
            ins=[cwin.opt()],
            outs=[cwout.opt()],
        )

    # identity for PE transposes / residual accumulation
    ident_f = consts.tile([128, 128], F32, tag="identf", name="identf")
    make_identity(nc, ident_f)
    ident = consts.tile([128, 128], BF16, tag="ident", name="ident")
    nc.vector.tensor_copy(out=ident, in_=ident_f)

    # ---- phase A0 DMAs: 2 groups x 16 contraction slices; 4-8KB/partition
    # transfers keep the DMA packets large ----
    NG, JG = 4, 8
    wts = []   # held weight group tiles, reused for batch1
    a0 = []    # batch0 activation group tiles
    for g in range(NG):
        wq_c = wpool.tile([128, JG, D], FP8, tag=f"wq{g}", name=f"wq{g}")
        wk_c = wpool.tile([128, JG, D], FP8, tag=f"wk{g}", name=f"wk{g}")
        wv_c = wpool.tile([128, JG, D], BF16, tag=f"wv{g}", name=f"wv{g}")
        nc.sync.dma_start(out=wq_c, in_=io["wq"][g])
        nc.scalar.dma_start(out=wk_c, in_=io["wk"][g])
        nc.gpsimd.dma_start(out=wv_c, in_=io["wv"][g])
        wts.append((wq_c, wk_c, wv_c))
        qc = apool.tile([128, JG, 128], FP8, tag=f"qc0_{g}", name="qc0")
        kc = apool.tile([128, JG, 128], FP8, tag=f"kc0_{g}", name="kc0")
        vc = apool.tile([128, JG, 128], BF16, tag=f"vc0_{g}", name="vc0")
        nc.sync.dma_start(out=qc, in_=io["qT"][0, g])
        nc.scalar.dma_start(out=kc, in_=io["kT"][0, g])
        nc.gpsimd.dma_start(out=vc, in_=io["vT"][0, g])
        a0.append((qc, kc, vc))

    # ================= phase A0: batch0 projections =================
    ps_proj0_ctx = tc.tile_pool(name="ps_proj0", bufs=1, space="PSUM")
    ps_proj0 = ps_proj0_ctx.__enter__()
    warm = ps_proj0.tile([128, 128], BF16, tag="warm", name="warm")
    nc.tensor.transpose(warm[:, :], ident[:, :], ident[:, :])
    Qp0 = ps_proj0.tile([128, D], F32, tag="Qp0", name="Qp0")
    Kp0 = ps_proj0.tile([128, D], F32, tag="Kp0", name="Kp0")
    Vp0 = ps_proj0.tile([128, D], F32, tag="Vp0", name="Vp0")

    for g in range(NG):
        qc, kc, vc = a0[g]
        wq_c, wk_c, wv_c = wts[g]
        for j in range(JG):
            st = g == 0 and j == 0
            sp = g == NG - 1 and j == JG - 1
            nc.tensor.matmul(Qp0[:, :], qc[:, j, :], wq_c[:, j, :], start=st, stop=sp)
            nc.tensor.matmul(Kp0[:, :], kc[:, j, :], wk_c[:, j, :], start=st, stop=sp)
            nc.tensor.matmul(Vp0[:, :], vc[:, j, :], wv_c[:, j, :], start=st, stop=sp)

    # batch1 activation streams: all on the sync queue, after the prefetches.
    # The a1pool rotation gives the later descgens WAR waits; sync's only
    # later work is the output stores, so the blocking is harmless there.
    a1 = []
    for g in range(NG):
        qc = a1pool.tile([128, JG, 128], FP8, tag=f"qc1_{g}", name="qc1")
        kc = a1pool.tile([128, JG, 128], FP8, tag=f"kc1_{g}", name="kc1")
        vc = a1pool.tile([128, JG, 128], BF16, tag=f"vc1_{g}", name="vc1")
        nc.sync.dma_start(out=qc, in_=io["qT"][1, g])
        nc.sync.dma_start(out=kc, in_=io["kT"][1, g])
        nc.sync.dma_start(out=vc, in_=io["vT"][1, g])
        a1.append((qc, kc, vc))

    # fc weights / residual / stats constants stream right after phase A0
    # (scalar/gpsimd queues; none of these DMAs carries a WAR wait, so the
    # issuing engines never block on them)
    wfcts = []
    for pt in range(NPC):
        wfct = fcpool.tile([128, 4, 512], BF16, tag=f"wfct{pt}", name=f"wfct{pt}")
        (nc.sync if pt % 2 == 0 else nc.gpsimd).dma_start(out=wfct, in_=io["wfc"][pt])
        wfcts.append(wfct)
    resid = []
    for b in range(BPC):
        t = fcpool.tile([128, NPC, 512], BF16, tag=f"resid{b}", name=f"resid{b}")
        (nc.sync if b == 0 else nc.gpsimd).dma_start(out=t, in_=io["resid"][b])
        resid.append(t)
    colsum = consts.tile([128, 4, NPC], BF16, tag="colsum", name="colsum")
    nc.gpsimd.dma_start(out=colsum, in_=io["colsum"][:, :, :])
    vsums = consts.tile([128, 2 * NPC], F32, tag="vsums", name="vsums")
    nc.gpsimd.dma_start(out=vsums, in_=io["vsums"][:, :])
    bng = consts.tile([128, 1], F32, tag="bng", name="bng")
    bnb = consts.tile([128, 1], F32, tag="bnb", name="bnb")
    nc.gpsimd.dma_start(out=bng, in_=io["bng"][:, :])
    nc.gpsimd.dma_start(out=bnb, in_=io["bnb"][:, :])

    # ---- evacuate batch0 Q/K/V, free proj0 banks ----
    Q_sb0 = sb.tile([128, D], BF16, tag="Q_sb", name="Q_sb0")
    K_sb0 = sb.tile([128, D], BF16, tag="K_sb", name="K_sb0")
    V_sb0 = sb.tile([128, D], BF16, tag="V_sb", name="V_sb0")
    nc.vector.tensor_copy(out=Q_sb0, in_=Qp0[:, :])
    nc.scalar.copy(out=K_sb0, in_=Kp0[:, :])
    nc.vector.tensor_copy(out=V_sb0, in_=Vp0[:, :])
    ps_proj0_ctx.__exit__(None, None, None)

    # attention-era PSUM (sm x3 + O0/O1) + batch1 projection banks (x3) = 8
    ps_s = ctx.enter_context(tc.tile_pool(name="ps_s", bufs=3, space="PSUM"))
    ps_o = ctx.enter_context(tc.tile_pool(name="ps_o", bufs=1, space="PSUM"))
    ps_proj1_ctx = tc.tile_pool(name="ps_proj1", bufs=1, space="PSUM")
    ps_proj1 = ps_proj1_ctx.__enter__()
    Qp1 = ps_proj1.tile([128, D], F32, tag="Qp1", name="Qp1")
    Kp1 = ps_proj1.tile([128, D], F32, tag="Kp1", name="Kp1")
    Vp1 = ps_proj1.tile([128, D], F32, tag="Vp1", name="Vp1")

    Opsums = [ps_o.tile([128, D], F32, tag=f"O{b}", name=f"O{b}") for b in range(BPC)]
    Oscs = [big.tile([128, D], F32, tag=f"Osc{b}", name=f"Osc{b}") for b in range(BPC)]

    def qk_transposes(Q_sb, K_sb, QT_sb, KT_sb):
        # copies all on Vector: Scalar's queue carries Exp/silu chains whose
        # latency must not gate the next batch's S-matmuls
        for src, dst in ((Q_sb, QT_sb), (K_sb, KT_sb)):
            for dc in range(4):
                tp = ps_s.tile([128, 128], BF16, tag="sm", name="stp")
                nc.tensor.transpose(tp[:, :], src[:, dc * 128:(dc + 1) * 128], ident[:, :])
                nc.vector.tensor_copy(out=dst[:, dc * 128:(dc + 1) * 128], in_=tp[:, :])

    def start_head(b, h, QT_sb, KT_sb):
        """Issue S-matmul + Exp for head h (finished one iteration later, so
        the Exp latency hides under the next head's S + filler PE work)."""
        po = (h % 2) * 64
        fo = (h // 2) * 128
        S = ps_s.tile([128, 128], F32, tag="sm", name="S")
        nc.tensor.matmul(S[:, :], QT_sb[po:po + 64, fo:fo + 128],
                         KT_sb[po:po + 64, fo:fo + 128], start=True, stop=True)
        e_f = sb.tile([128, 128], BF16, tag="e_f", name="e_f")
        lsum = small.tile([128, 1], F32, tag="lsum", name="lsum")
        nc.scalar.activation(out=e_f, in_=S[:, :], func=AF.Exp,
                             scale=1.0 / (QSCALE * KVSCALE), accum_out=lsum)
        rs = small.tile([128, 1], F32, tag="rs", name="rs")
        nc.vector.reciprocal(rs, lsum)
        return (h, e_f, rs)

    def finish_head(b, st, V_sb):
        h, e_f, rs = st
        tpa = ps_s.tile([128, 128], BF16, tag="sm", name="stp")
        nc.tensor.transpose(tpa[:, :], e_f[:, :], ident[:, :])
        aT = sb.tile([128, 128], BF16, tag="aT", name="aT")
        nc.vector.tensor_copy(out=aT, in_=tpa[:, :])
        nc.tensor.matmul(Opsums[b][:, h * 64:(h + 1) * 64], aT[:, :],
                         V_sb[:, h * 64:(h + 1) * 64], start=True, stop=True)
        nc.vector.tensor_scalar_mul(out=Oscs[b][:, h * 64:(h + 1) * 64],
                                    in0=Opsums[b][:, h * 64:(h + 1) * 64],
                                    scalar1=rs)

    def head_loop(b, QT_sb, KT_sb, V_sb, filler):
        pending = None
        for h in range(NH):
            st = start_head(b, h, QT_sb, KT_sb)
            if filler is not None:
                filler(h)
            if pending is not None:
                finish_head(b, pending, V_sb)
            pending = st
        finish_head(b, pending, V_sb)

    # ---- batch0 attention, with batch1 projections as PE filler ----
    QT0 = sb.tile([128, D], BF16, tag="QT_sb", name="QT0")
    KT0 = sb.tile([128, D], BF16, tag="KT_sb", name="KT0")
    qk_transposes(Q_sb0, K_sb0, QT0, KT0)

    def proj1_filler(h):
        g = h // 2
        qc, kc, vc = a1[g]
        wq_c, wk_c, wv_c = wts[g]
        for j in range((h % 2) * 4, (h % 2) * 4 + 4):
            st = h == 0 and j == 0
            sp = h == NH - 1 and j == JG - 1
            nc.tensor.matmul(Qp1[:, :], qc[:, j, :], wq_c[:, j, :], start=st, stop=sp)
            nc.tensor.matmul(Kp1[:, :], kc[:, j, :], wk_c[:, j, :], start=st, stop=sp)
            nc.tensor.matmul(Vp1[:, :], vc[:, j, :], wv_c[:, j, :], start=st, stop=sp)

    head_loop(0, QT0, KT0, V_sb0, proj1_filler)

    # ---- evacuate batch1 Q/K/V, free proj1 banks; open fc-era pools ----
    Q_sb1 = sb.tile([128, D], BF16, tag="Q_sb", name="Q_sb1")
    K_sb1 = sb.tile([128, D], BF16, tag="K_sb", name="K_sb1")
    V_sb1 = sb.tile([128, D], BF16, tag="V_sb", name="V_sb1")
    nc.vector.tensor_copy(out=Q_sb1, in_=Qp1[:, :])
    nc.scalar.copy(out=K_sb1, in_=Kp1[:, :])
    nc.vector.tensor_copy(out=V_sb1, in_=Vp1[:, :])
    ps_proj1_ctx.__exit__(None, None, None)
    ps_fc = ctx.enter_context(tc.tile_pool(name="ps_fc", bufs=2, space="PSUM"))
    ps_fs = ctx.enter_context(tc.tile_pool(name="ps_fs", bufs=1, space="PSUM"))
    fcS = ps_fs.tile([128, 2 * NPC], F32, tag="fcS", name="fcS")

    def rsqrt_dve(dst, src, uid, iters=2):
        """1/sqrt(src) on the Vector engine: bit-trick seed + Newton iters.
        No Scalar sqrt -> no ACT table-set switch. rel err ~4e-6 (2 it),
        ~1.8e-3 (1 it)."""
        xi = small.tile([128, 1], I32, tag="nxi", name="nxi")
        nc.vector.tensor_scalar(out=xi, in0=src.bitcast(I32), scalar1=1,
                                scalar2=None, op0=ALU.logical_shift_right)
        yi = small.tile([128, 1], I32, tag="nyi", name="nyi")
        nc.vector.tensor_scalar(out=yi, in0=xi, scalar1=-1, scalar2=0x5F3759DF,
                                op0=ALU.mult, op1=ALU.add)
        y = small.tile([128, 1], F32, tag="ny0", name="ny0")
        nc.vector.tensor_copy(out=y, in_=yi.bitcast(F32))
        for it in range(iters):
            t = small.tile([128, 1], F32, tag=f"nt{it}", name="nt")
            nc.vector.tensor_mul(out=t, in0=y, in1=y)
            nc.vector.tensor_mul(out=t, in0=t, in1=src)
            nc.vector.tensor_scalar(out=t, in0=t, scalar1=-0.5, scalar2=1.5,
                                    op0=ALU.mult, op1=ALU.add)
            yn = dst if it == iters - 1 else small.tile([128, 1], F32, tag="ny1", name="ny1")
            nc.vector.tensor_mul(out=yn, in0=y, in1=t)
            y = yn

    def silu_ln_xhat(b):
        """silu (exp + DVE) -> LayerNorm -> xhat (no PE work)."""
        Osc = Oscs[b]
        # silu(x) = x * 0.5*(1 + tanh(x/2)); Tanh shares the exp ACT table
        # set (no reload) and the elementwise ops run on the idle GpSimd
        e1 = big.tile([128, D], F32, tag="e1", name="e1", bufs=2)
        nc.scalar.activation(out=e1, in_=Osc, func=AF.Tanh, scale=0.5)
        nc.vector.tensor_scalar(out=e1, in0=e1, scalar1=0.5, scalar2=0.5,
                                op0=ALU.mult, op1=ALU.add)
        nc.vector.tensor_mul(out=Osc, in0=Osc, in1=e1)   # Osc <- silu(Osc)
        st6 = small.tile([128, 6], F32, tag="st6", name="st6")
        nc.vector.bn_stats(out=st6, in_=Osc)
        mv = small.tile([128, 2], F32, tag="mv", name="mv")
        nc.vector.bn_aggr(out=mv, in_=st6)
        vu = small.tile([128, 1], F32, tag="vu", name="vu")
        nc.vector.tensor_scalar_mul(out=vu, in0=mv[:, 1:2], scalar1=float(D) / (D - 1))
        rstd = small.tile([128, 1], F32, tag="rstd", name="rstd")
        rsqrt_dve(rstd, vu, b)
        xhat = sb.tile([128, D], BF16, tag="xhat", name="xhat")
        nc.vector.tensor_scalar(out=xhat, in0=Osc, scalar1=mv[:, 0:1], scalar2=rstd,
                                op0=ALU.subtract, op1=ALU.mult)
        return xhat

    def xt_fcs(b, xhat):
        """xhat^T via PE transposes + the per-pt fc colsum matmuls."""
        xT = sb.tile([128, D], BF16, tag="xT", name="xT")
        for dc in range(4):
            tp = ps_s.tile([128, 128], BF16, tag="sm", name="stp")
            nc.tensor.transpose(tp[:, :], xhat[:, dc * 128:(dc + 1) * 128], ident[:, :])
            nc.vector.tensor_copy(out=xT[:, dc * 128:(dc + 1) * 128], in_=tp[:, :])
        for dc in range(4):
            nc.tensor.matmul(fcS[:, b * NPC:(b + 1) * NPC],
                             xT[:, dc * 128:(dc + 1) * 128],
                             colsum[:, dc, :], start=dc == 0, stop=dc == 3)
        return xT

    segs = [big.tile([128, NPC, 512], BF16, tag=f"seg{b}", name=f"seg{b}")
            for b in range(BPC)]
    pcols = stat.tile([128, 16], F32, tag="pcols", name="pcols")

    def fc_pt(b, pt, xT):
        """fc for one 512-pixel tile + residual + sumsq accumulation."""
        O2 = ps_fc.tile([128, 512], F32, tag="O2", name="O2")
        for dc in range(4):
            nc.tensor.matmul(O2[:, :], xT[:, dc * 128:(dc + 1) * 128],
                             wfcts[pt][:, dc, :], start=dc == 0, stop=False)
        nc.tensor.matmul(O2[:, :], ident[:, :], resid[b][:, pt, :],
                         start=False, stop=True)
        seg = segs[b][:, pt, :]
        if pt % 2 == 0:
            nc.vector.tensor_copy(out=seg, in_=O2[:, :])
        else:
            nc.scalar.copy(out=seg, in_=O2[:, :])
        junk = sb.tile([128, 512], BF16, tag="junk", name="junk")
        nc.scalar.activation(out=junk, in_=O2[:, :], func=AF.Square,
                             accum_out=pcols[:, b * NPC + pt:b * NPC + pt + 1])

    # ---- batch1 Q/K transposes first (PE/Vector free), then batch0's
    # silu/LN chain (Scalar/DVE) overlaps batch1's first attention heads;
    # batch0's xT generation and fc tiles fill the PE inside the head loop ----
    QT1 = sb.tile([128, D], BF16, tag="QT_sb", name="QT1")
    KT1 = sb.tile([128, D], BF16, tag="KT_sb", name="KT1")
    qk_transposes(Q_sb1, K_sb1, QT1, KT1)
    xhat0 = silu_ln_xhat(0)
    xT0_box = []

    def fc0_filler(h):
        if h == 0:
            xT0_box.append(xt_fcs(0, xhat0))
        else:
            fc_pt(0, h - 1, xT0_box[0])

    head_loop(1, QT1, KT1, V_sb1, fc0_filler)
    fc_pt(0, NPC - 1, xT0_box[0])

    # ---- batch1 silu/LN/xT + fc ----
    xhat1 = silu_ln_xhat(1)
    xT1 = xt_fcs(1, xhat1)
    for pt in range(NPC):
        fc_pt(1, pt, xT1)

    # second warm AllReduce, emitted HERE so its trigger (which blocks the
    # gpsimd queue until warm #1 completes) sits after every descgen the
    # compute phases need; it still executes mid-kernel, re-syncing cores
    for _ in range(max(0, N_WARM_AR - 1) if not SKIP_COLL else 0):
        nc.gpsimd.collective_compute(
            "AllReduce",
            ALU.add,
            replica_groups=[list(range(N_CORES))],
            ins=[cwin.opt()],
            outs=[cwout.opt()],
        )

    # ---- BN stats AllReduce + normalize + store ----
    stats2 = stat.tile([128, 2], F32, tag="stats2", name="stats2")
    ssum = stat.tile([128, NPC], F32, tag="ssum", name="ssum")
    nc.vector.tensor_add(out=ssum, in0=vsums[:, 0:NPC], in1=vsums[:, NPC:2 * NPC])
    nc.vector.tensor_add(out=ssum, in0=ssum, in1=fcS[:, 0:NPC])
    nc.vector.tensor_add(out=ssum, in0=ssum, in1=fcS[:, NPC:2 * NPC])
    nc.vector.reduce_sum(stats2[:, 0:1], ssum, axis=AX.X)
    nc.vector.reduce_sum(stats2[:, 1:2], pcols[:, 0:16], axis=AX.X)

    cin = dram.tile([128, 2], F32, tag="cin", name="cin")
    cout = dram.tile([128, 2], F32, tag="cout", name="cout")
    nc.gpsimd.dma_start(out=cin[:, :], in_=stats2)
    if SKIP_COLL:
        nc.gpsimd.dma_start(out=cout[:, :], in_=cin[:, :])
    else:
        nc.gpsimd.collective_compute(
            "AllReduce",
            ALU.add,
            replica_groups=[list(range(N_CORES))],
            ins=[cin.opt()],
            outs=[cout.opt()],
        )
    red = stat.tile([128, 2], F32, tag="red", name="red")
    nc.gpsimd.dma_start(out=red[:, :], in_=cout[:, :])

    inv_n = 1.0 / float(B * P)
    me = stat.tile([128, 2], F32, tag="me", name="me")  # [mean, E[x^2]]
    nc.vector.tensor_scalar_mul(out=me, in0=red[:, :], scalar1=inv_n)
    msq = small.tile([128, 1], F32, tag="msq", name="msq")
    nc.vector.tensor_mul(out=msq, in0=me[:, 0:1], in1=me[:, 0:1])
    var = small.tile([128, 1], F32, tag="var", name="var")
    nc.vector.tensor_sub(out=var, in0=me[:, 1:2], in1=msq)
    nc.vector.tensor_scalar_add(out=var, in0=var, scalar1=BN_EPS)
    invs = small.tile([128, 1], F32, tag="invs", name="invs")
    rsqrt_dve(invs, var, 99, iters=1)
    scl = small.tile([128, 1], F32, tag="scl", name="scl")
    nc.vector.tensor_mul(out=scl, in0=bng, in1=invs)
    tmp = small.tile([128, 1], F32, tag="tmp", name="tmp")
    nc.vector.tensor_mul(out=tmp, in0=me[:, 0:1], in1=scl)
    shf = small.tile([128, 1], F32, tag="shf", name="shf")
    nc.vector.tensor_sub(out=shf, in0=bnb, in1=tmp)

    # normalize all segments (Vector/Scalar split), then one large
    # contiguous store per batch (8KB/partition -> big DMA packets)
    for b in range(BPC):
        for pt in range(NPC):
            seg = segs[b][:, pt, :]
            if pt % 2 == 0:
                nc.vector.tensor_scalar(out=seg, in0=seg, scalar1=scl, scalar2=shf,
                                        op0=ALU.mult, op1=ALU.add)
            else:
                nc.scalar.activation(out=seg, in_=seg, func=AF.Identity,
                                     scale=scl, bias=shf)
        (nc.sync if b == 0 else nc.gpsimd).dma_start(out=io["out"][b], in_=segs[b][:, :, :])


def _build():
    key = (SKIP_COLL, N_WARM_AR, USE_LNEXP)
    if key in _BUILD_CACHE:
        return _BUILD_CACHE[key]
    nc = bacc.Bacc("TRN2", target_bir_lowering=False, debug=False, num_devices=N_CORES)
    io = {
        "qT": nc.dram_tensor("qT", [BPC, 4, 128, 8, 128], FP8, kind="ExternalInput").ap(),
        "kT": nc.dram_tensor("kT", [BPC, 4, 128, 8, 128], FP8, kind="ExternalInput").ap(),
        "vT": nc.dram_tensor("vT", [BPC, 4, 128, 8, 128], BF16, kind="ExternalInput").ap(),
        "resid": nc.dram_tensor("resid", [BPC, C, NPC, 512], BF16, kind="ExternalInput").ap(),
        "wq": nc.dram_tensor("wq", [4, 128, 8, D], FP8, kind="ExternalInput").ap(),
        "wk": nc.dram_tensor("wk", [4, 128, 8, D], FP8, kind="ExternalInput").ap(),
        "wv": nc.dram_tensor("wv", [4, 128, 8, D], BF16, kind="ExternalInput").ap(),
        "wfc": nc.dram_tensor("wfc", [NPC, 128, 4, 512], BF16, kind="ExternalInput").ap(),
        "colsum": nc.dram_tensor("colsum", [128, 4, NPC], BF16, kind="ExternalInput").ap(),
        "vsums": nc.dram_tensor("vsums", [C, 2 * NPC], F32, kind="ExternalInput").ap(),
        "bng": nc.dram_tensor("bng", [C, 1], F32, kind="ExternalInput").ap(),
        "bnb": nc.dram_tensor("bnb", [C, 1], F32, kind="ExternalInput").ap(),
        "out": nc.dram_tensor("out", [BPC, C, P], BF16, kind="ExternalOutput").ap(),
    }
    from contextlib import ExitStack
    with tile.TileContext(nc) as tc, ExitStack() as ctx:
        _emit(ctx, nc, tc, io)
    nc.compile()
    _BUILD_CACHE[key] = nc
    return nc


def _bf16(x):
    import ml_dtypes
    return np.ascontiguousarray(np.asarray(x, np.float32).astype(ml_dtypes.bfloat16))


def _fp8(x):
    import ml_dtypes
    dt = ml_dtypes.float8_e4m3fn if os.environ.get('BASS_FP8','1')=='1' else ml_dtypes.bfloat16
    return np.ascontiguousarray(np.asarray(x, np.float32).astype(dt))


def _pack_acts(xT):
    # [b, 4096, 128] -> [b, 4, 128, 8, 128]  (group, partition, slice, c)
    b = xT.shape[0]
    return np.ascontiguousarray(
        xT.reshape(b, 4, 8, 128, 128).transpose(0, 1, 3, 2, 4))


def _pack_w(w):
    # [4096, D] -> [4, 128, 8, D]
    return np.ascontiguousarray(w.reshape(4, 8, 128, -1).transpose(0, 2, 1, 3))


def kernel(v, k, q, w_qs, w_ks, w_vs, w_fc, ln_gamma, ln_beta, temperature,
           bn_gamma, bn_beta, **_ignored):
    v = np.asarray(v, np.float32)
    k = np.asarray(k, np.float32)
    q = np.asarray(q, np.float32)
    w_qs = np.asarray(w_qs, np.float32)
    w_ks = np.asarray(w_ks, np.float32)
    w_vs = np.asarray(w_vs, np.float32)
    w_fc = np.asarray(w_fc, np.float32)
    ln_gamma = np.asarray(ln_gamma, np.float32)
    ln_beta = np.asarray(ln_beta, np.float32)
    temp = float(np.asarray(temperature))
    bn_gamma = np.asarray(bn_gamma, np.float32)
    bn_beta = np.asarray(bn_beta, np.float32)

    qf = q.reshape(B, C, P)
    kf = k.reshape(B, C, P)
    vf = v.reshape(B, C, P)
    qT = _fp8(_pack_acts(qf.transpose(0, 2, 1)))
    kT = _fp8(_pack_acts(kf.transpose(0, 2, 1)))
    vT = _bf16(_pack_acts(vf.transpose(0, 2, 1)))
    wq = _fp8(_pack_w((w_qs * (QSCALE / temp)).T))
    wk = _fp8(_pack_w((w_ks * KVSCALE).T))
    wv = _bf16(_pack_w(w_vs.T))
    # wfc packed as [pt, d_low, dc, p_in_pt]: wfcT_eff[dc*128+d_low, pt*512+p]
    wfcT_eff = (w_fc * ln_gamma[None, :]).T  # [D, P]
    wfc = _bf16(wfcT_eff.reshape(4, 128, NPC, 512).transpose(2, 1, 0, 3))
    # colsum[d_low, dc, pt] = sum_{p in pt} wfcT_eff[dc*128+d_low, pt*512+p]
    colsum = _bf16(wfcT_eff.reshape(4, 128, NPC, 512).sum(-1).transpose(1, 0, 2))
    bias_fc = (w_fc @ ln_beta).astype(np.float32)
    veff = vf + bias_fc[None, None, :]                      # [B, C, P] f32
    resid = _bf16(veff.reshape(B, C, NPC, 512))
    # per-(b,pt) channel sums of the (bf16-rounded) residual, f32 accumulated
    vsums_full = np.asarray(resid, np.float32).sum(-1)      # [B, C, NPC]
    bng = np.ascontiguousarray(bn_gamma.reshape(C, 1))
    bnb = np.ascontiguousarray(bn_beta.reshape(C, 1))

    nc = _build()
    in_maps = []
    for i in range(N_CORES):
        bs = slice(BPC * i, BPC * (i + 1))
        vsums = np.ascontiguousarray(
            vsums_full[bs].transpose(1, 0, 2).reshape(C, 2 * NPC))
        in_maps.append({
            "qT": qT[bs], "kT": kT[bs], "vT": vT[bs], "resid": resid[bs],
            "wq": wq, "wk": wk, "wv": wv, "wfc": wfc,
            "colsum": colsum, "vsums": vsums,
            "bng": bng, "bnb": bnb,
        })
    res = run_bass_kernel_spmd(nc, in_maps, core_ids=list(range(N_CORES)))
    global LAST_RESULTS
    LAST_RESULTS = res
    out = np.concatenate([np.asarray(res.results[i]["out"], np.float32)
                          for i in range(N_CORES)], axis=0)   # [B, C, P]
    return out.reshape(B, C, HH, WW)
